# revision 35
# baseline (speedup 1.0000x reference)
"""Trainium2 Bass kernel for nn_Divergence2d.

Math (from the reference):
  q = C//4 = 4 channel groups A=x[:, :4], B=x[:,4:8], C=x[:,8:12], D=x[:,12:16]
  With per-group channel sums  A(r,c) = sum_ch lam_ch x[ch, r, c]  (lam only
  for group A) and a padded map  Gpad[r, c] = G[r-2, c-2]:

    out1[i,j] = (Apad[i+1, j+2] - Apad[i+1, j]) + Bpad[i, j+1] - Bpad[i+2, j+1]
    out2[i,j] = (Cpad[i+1, j+2] - Cpad[i+1, j]) + Dpad[i, j+1] - Dpad[i+2, j+1]

  for i,j in [0, 514)  (lam applied in the DVE combine when all lams equal).

Strategy: pure data parallel, 2 images per core on 8 cores.  Per image the
514 output rows are split into 4 full blocks of 126 plus a 10-row runt;
per full block ONE HWDGE DMA loads a row window of all 16 channels into an
SBUF tile [rows, 16ch x 512] (rhs pool 5-deep, so the load stream rides
out transient HBM-contention stalls).  The TensorE does the stencil via
stationary matrices (weights [window_row, out_row], channel sum by PSUM
accumulation over 4 chained matmuls).  All matmuls run in float32r mode;
the loose rel-err budget (2e-2) dwarfs the ~1e-3 rounding.

The runt uses a (row, channel)-packed tile (partition r*4+c4, one column
block per output map) with host-precomputed [128, 40] weights ("rw" input)
so its 320 KB spread over 10 DMA engines / 5 SBUF ports in one K=40 matmul
per map — a 10-partition layout would funnel through 2 SBUF AXI ports at
~54 GB/s and head-of-line block the load FIFO of engines 0-9 for ~6 us.
Image 0 processes it second (so block 0 leads the startup ramp), image 1
first (so the 4 full blocks stream uniformly into the drain).

Vertical conv padding is folded into the weights instead of rhs memsets:
  - block 0 loads x rows [0:128) at partition 0 and uses row-shifted
    weights (taps at negative x rows simply have no weight entry);
  - runt taps at x rows >= 512 have no weight entry in "rw".
ScalarE (ACT) drains PSUM into zero-padded SBUF staging tiles (pad columns
memset only on each staging buffer's first use); DVE does the combine ops;
per block the ch0 store rides the scalar/Act HWDGE ring and the ch1 store
the gpsimd SWDGE path — two independent trigger streams keep the write
traffic from backing up behind the ACT drains.  The final block's load is
group-split and its out2 combine + store row-split, shortening the
end-of-kernel drain.
"""
import sys

for _p in (
    "/root/.axon_site",
    "/root/.axon_site/_ro/trn_rl_repo",
    "/root/.axon_site/_ro/pypackages",
    "/opt/trn_rl_repo",
):
    if _p not in sys.path:
        sys.path.append(_p)

import numpy as np

N_CORES = 8
N, C, H, W = 16, 16, 512, 512
PB = N // N_CORES          # images per core
HO = WO = H + 2            # 514
CWPAD = 516                # staging width (2-col pad each side)
BLK = 126                  # output rows per block (matmul M)
BLOCKS = []
_i0 = 0
while _i0 < HO:
    BLOCKS.append((_i0, min(BLK, HO - _i0)))
    _i0 += BLK
# -> [(0,126), (126,126), (252,126), (378,126), (504,10)]

_cache = {}


def _build(lam4):
    import concourse.bacc as bacc
    import concourse.mybir as mybir
    from concourse.tile import TileContext

    f32 = mybir.dt.float32
    f32r = mybir.dt.float32r
    ALU = mybir.AluOpType
    ACT_COPY = mybir.ActivationFunctionType.Copy
    lam_eq = all(float(v) == float(lam4[0]) for v in lam4)

    nc = bacc.Bacc("TRN2", target_bir_lowering=False, debug=False,
                   num_devices=N_CORES, detect_race_conditions=False)
    x = nc.dram_tensor("x", (PB, C, H, W), f32, kind="ExternalInput")
    rw = nc.dram_tensor("rw", (128, 40), f32, kind="ExternalInput")
    out = nc.dram_tensor("out", (PB, 2, HO, WO), f32, kind="ExternalOutput")

    with TileContext(nc) as tc:
        with (
            tc.tile_pool(name="consts", bufs=1) as c_pool,
            tc.tile_pool(name="rhs", bufs=5) as rhs_pool,
            tc.tile_pool(name="rrhs", bufs=1) as rr_pool,
            tc.tile_pool(name="psum", bufs=2, space="PSUM") as ps_pool,
            tc.tile_pool(name="stage", bufs=2) as st_pool,
            tc.tile_pool(name="outs", bufs=3) as out_pool,
            tc.tile_pool(name="dtmp", bufs=2) as d_pool,
        ):
            # ---- one-time stencil weights [128 window rows, 126 out rows] --
            # Interior blocks (window row r = x row i0-2+r, out local m):
            #   S_s1[r, m] = d(r, m+1)           (A/C tap at x row i-1)
            #   S_bd[r, m] = d(r, m) - d(r, m+2) (B/D rows i-2 / i)
            # Block 0 (window row r = x row r):
            #   S_s1f[r, m] = d(r, m-1)
            #   S_bdf[r, m] = d(r, m-2) - d(r, m)
            def iota_t(tag, base, mult):
                t = c_pool.tile([128, BLK], f32, tag=tag, name=tag)
                nc.gpsimd.iota(t[:, :], pattern=[[0 if mult else 1, BLK]],
                               base=base, channel_multiplier=mult,
                               allow_small_or_imprecise_dtypes=True)
                return t

            R0 = iota_t("R0", 0, 1)          # r
            R1 = iota_t("R1", 1, 1)          # r + 1
            R2 = iota_t("R2", 2, 1)          # r + 2
            Sm0 = iota_t("Sm0", 0, 0)        # m
            Sm1 = iota_t("Sm1", 1, 0)        # m + 1
            Sm2 = iota_t("Sm2", 2, 0)        # m + 2

            def eq_t(tag, a, b, dt=None):
                t = c_pool.tile([128, BLK], dt or f32, tag=tag, name=tag)
                nc.vector.tensor_tensor(t[:, :], a[:, :], b[:, :], ALU.is_equal)
                return t

            # final weight tiles are float32r so their producing instruction
            # "rounds" them for the fp32r matmuls (values are exact anyway)
            S_s1 = eq_t("S_s1", R0, Sm1, f32r)
            e0 = eq_t("e0", R0, Sm0)         # (r == m)
            e2 = eq_t("e2", R0, Sm2)         # (r == m+2)
            S_bd = c_pool.tile([128, BLK], f32r, tag="S_bd")
            nc.vector.tensor_tensor(S_bd[:, :], e0[:, :], e2[:, :], ALU.subtract)
            S_s1f = eq_t("S_s1f", R1, Sm0, f32r)   # (r == m-1)
            e0f = eq_t("e0f", R2, Sm0)       # (r == m-2)
            S_bdf = c_pool.tile([128, BLK], f32r, tag="S_bdf")
            nc.vector.tensor_tensor(S_bdf[:, :], e0f[:, :], e0[:, :], ALU.subtract)

            if lam_eq:
                S_A_per_ch = [S_s1] * 4      # lam applied in the combine
                S_Af_per_ch = [S_s1f] * 4
            else:
                S_A_per_ch, S_Af_per_ch = [], []
                for c4 in range(4):
                    t = c_pool.tile([128, BLK], f32r, tag=f"S_A{c4}",
                                    name=f"S_A{c4}")
                    nc.vector.tensor_scalar_mul(t[:, :], S_s1[:, :],
                                                float(lam4[c4]))
                    S_A_per_ch.append(t)
                    tf = c_pool.tile([128, BLK], f32r, tag=f"S_Af{c4}",
                                     name=f"S_Af{c4}")
                    nc.vector.tensor_scalar_mul(tf[:, :], S_s1f[:, :],
                                                float(lam4[c4]))
                    S_Af_per_ch.append(tf)

            # runt weights, host-precomputed (see _runt_weights): one tile
            # [128, 40], rows c4*10+r, columns g*10+m (g = map A,B,C,D) —
            # a single K=40 matmul per output map.
            WRt = c_pool.tile([128, 40], f32r, tag="WRt", name="WRt")
            nc.sync.dma_start(out=WRt[:, :], in_=rw[:, :].bitcast(f32r))

            # ---- shared per-block tail: PSUM drain, combine, store ------
            def finish_block(n, i0, nr, ps, prime, tail_split=False):
                # ACT drains the A/C maps into zero-padded staging tiles
                # (pad columns memset only on each buffer's first use);
                # B/D are read directly from PSUM in the DVE combine
                st = {}
                for g in (0, 2):      # A/C: data at cols [2:514)
                    s = st_pool.tile([128, CWPAD], f32, tag=f"st{g}",
                                     name=f"st{g}")
                    st[g] = s
                    if prime:
                        nc.vector.memset(s[:, 0:2], 0.0)
                        nc.vector.memset(s[:, 514:CWPAD], 0.0)
                    nc.scalar.activation(s[0:nr, 2:514], ps[g][0:nr, :],
                                         ACT_COPY)
                # DVE combine
                o = out_pool.tile([128, 2 * WO], f32, tag="o")
                dA = d_pool.tile([128, WO], f32, tag="dA")
                nc.vector.tensor_tensor(dA[0:nr, :], st[0][0:nr, 2:2 + WO],
                                        st[0][0:nr, 0:WO], ALU.subtract)
                if lam_eq:
                    lam0 = float(lam4[0])
                    nc.vector.tensor_scalar_mul(o[0:nr, 0:1],
                                                dA[0:nr, 0:1], lam0)
                    nc.vector.tensor_scalar_mul(o[0:nr, 513:514],
                                                dA[0:nr, 513:514], lam0)
                    nc.vector.scalar_tensor_tensor(
                        o[0:nr, 1:513], dA[0:nr, 1:513], lam0,
                        ps[1][0:nr, :], ALU.mult, ALU.add)
                else:
                    nc.vector.tensor_scalar_mul(o[0:nr, 0:1],
                                                dA[0:nr, 0:1], 1.0)
                    nc.vector.tensor_scalar_mul(o[0:nr, 513:514],
                                                dA[0:nr, 513:514], 1.0)
                    nc.vector.tensor_tensor(o[0:nr, 1:513],
                                            dA[0:nr, 1:513],
                                            ps[1][0:nr, :], ALU.add)
                nc.vector.tensor_tensor(o[0:nr, WO:2 * WO],
                                        st[2][0:nr, 2:2 + WO],
                                        st[2][0:nr, 0:WO], ALU.subtract)
                nc.vector.tensor_tensor(o[0:nr, WO + 1:WO + 513],
                                        o[0:nr, WO + 1:WO + 513],
                                        ps[3][0:nr, :], ALU.add)
                # store: one DMA per channel; the (w2, w) split merges back
                # into one 2056B full-row packet per out row
                osrc = o[0:nr, :].rearrange("p (ch w2 w) -> p ch w2 w",
                                            w2=2, w=WO // 2)
                ov = out[n].rearrange("ch r (w2 w) -> ch r w2 w",
                                      w=WO // 2)
                for ch in range(2):
                    nc.scalar.dma_start(out=ov[ch, i0:i0 + nr, :, :],
                                        in_=osrc[:, ch, :, :])

            # The 10-row runt (out rows 504-513) loads in a
            # (row, channel)-packed layout: x[4g+c4, 502+r, :] lands on
            # partition r*4+c4, column block g.  Its 320 KB then spread
            # over 5 SBUF AXI ports and 10 DMA engines (descriptor groups
            # are assigned to engines by the source AP's outer dim = r)
            # instead of funneling through 2 ports, which head-of-line
            # blocked engines 0-9's load FIFO for ~6 us.  One K=40 matmul
            # per map replaces the 4-chained-matmul group.
            def runt_block(n, prime):
                rt = rr_pool.tile([128, 4 * 512], f32r, tag="rt")
                for g in range(4):
                    col = g * 512
                    nc.sync.dma_start(
                        out=rt[0:40, col:col + 512],
                        in_=x[n, 4 * g:4 * g + 4, 502:512, :].rearrange(
                            "c r w -> r c w").bitcast(f32r))
                i0r, nrr = BLOCKS[-1]
                ps = {}
                for g in range(4):
                    col = g * 512
                    p = ps_pool.tile([128, 512], f32, tag=f"ps{g}",
                                     name=f"ps{g}")
                    ps[g] = p
                    nc.tensor.matmul(p[0:nrr, :],
                                     WRt[0:40, g * 10:g * 10 + 10],
                                     rt[0:40, col:col + 512],
                                     start=True, stop=True)
                finish_block(n, i0r, nrr, ps, prime)

            # ---- main loop ---------------------------------------------
            # BOTH images' runts are processed right after image 0's block
            # 0: block 0's load stream leads the startup ramp, the runts'
            # port-bound loads land early where the engines still have
            # slack, and everything after is a pure uniform full-block
            # stream straight into the end-of-kernel drain.
            blk_idx = 0
            for n in range(PB):
                # ---- 4 full blocks, uniform stream ----------------------
                for bi, (i0, nr) in enumerate(BLOCKS[:-1]):
                    blk_idx += 1
                    first = (i0 == 0)
                    if first:
                        rlo, K = 0, 128          # x rows [0:128) at part 0
                    else:
                        rlo = i0 - 2             # window row r = x row rlo+r
                        K = 128
                    t = rhs_pool.tile([128, 16 * 512], f32r, tag="rhs")
                    tv = t[:, :].rearrange("p (c w) -> p c w", w=512)
                    last_block = (n == PB - 1 and i0 == BLOCKS[-2][0])
                    if last_block:
                        # final processed block: split by channel groups so
                        # each map's matmuls start before the whole block is
                        # resident (shorter end-of-kernel drain).
                        # NOTE: splits must keep the full 128-partition extent
                        # (row-splits would engage only 2 of 16 SDMA engines)
                        nc.sync.dma_start(out=tv[0:K, 0:8, :],
                                          in_=x[n, 0:8, rlo:rlo + K, :].rearrange(
                                              "c r w -> r c w").bitcast(f32r))
                        nc.sync.dma_start(out=tv[0:K, 8:12, :],
                                          in_=x[n, 8:12, rlo:rlo + K, :].rearrange(
                                              "c r w -> r c w").bitcast(f32r))
                        nc.sync.dma_start(out=tv[0:K, 12:14, :],
                                          in_=x[n, 12:14, rlo:rlo + K, :].rearrange(
                                              "c r w -> r c w").bitcast(f32r))
                        nc.sync.dma_start(out=tv[0:K, 14:16, :],
                                          in_=x[n, 14:16, rlo:rlo + K, :].rearrange(
                                              "c r w -> r c w").bitcast(f32r))
                    elif blk_idx == 1:
                        # channel-wise head split: queues start draining after
                        # ~256 descriptors generate instead of 2048
                        nc.sync.dma_start(out=tv[0:K, 0:2, :],
                                          in_=x[n, 0:2, rlo:rlo + K, :].rearrange(
                                              "c r w -> r c w").bitcast(f32r))
                        nc.sync.dma_start(out=tv[0:K, 2:16, :],
                                          in_=x[n, 2:16, rlo:rlo + K, :].rearrange(
                                              "c r w -> r c w").bitcast(f32r))
                    else:
                        nc.sync.dma_start(out=tv[0:K, :, :],
                                          in_=x[n, :, rlo:rlo + K, :].rearrange(
                                              "c r w -> r c w").bitcast(f32r))
                    # group order A,B then C,D: the out1 combine only needs
                    # maps 0/1, so DVE overlaps the second half of the matmuls
                    if first:
                        groups = [(0, S_Af_per_ch), (1, [S_bdf] * 4),
                                  (2, [S_s1f] * 4), (3, [S_bdf] * 4)]
                    else:
                        groups = [(0, S_A_per_ch), (1, [S_bd] * 4),
                                  (2, [S_s1] * 4), (3, [S_bd] * 4)]
                    ps = {}
                    for g, weights in groups:
                        p = ps_pool.tile([128, 512], f32, tag=f"ps{g}",
                                         name=f"ps{g}")
                        ps[g] = p
                        for c4 in range(4):
                            ch = 4 * g + c4
                            nc.tensor.matmul(
                                p[0:BLK, :],
                                weights[c4][0:K, :],
                                t[0:K, 512 * ch:512 * ch + 512],
                                start=(c4 == 0), stop=(c4 == 3))
                    finish_block(n, i0, nr, ps, blk_idx <= 2,
                                 tail_split=last_block)
                    if n == 0 and bi == 0:
                        for nr_img in range(PB):
                            blk_idx += 1
                            runt_block(nr_img, blk_idx <= 2)
    nc.finalize()
    return nc


def _get_nc(lam4):
    key = tuple(float(v) for v in lam4)
    if key not in _cache:
        _cache[key] = _build(key)
    return _cache[key]


def _runt_weights(lam4):
    """[128, 40] f32: runt stencil weights, rows r*4+c4, cols g*10+m.

    g = 0..3 -> maps A (lam-scaled unless all lams equal; then scaled in
    the combine), B, C, D.  Out row 504+m: A/C tap at x row 503+m
    (r = m+1), B/D taps +x[502+m] (r = m) and -x[504+m] (r = m+2); taps
    with r >= 10 (x rows >= 512) drop.
    """
    lam_eq = all(float(v) == float(lam4[0]) for v in lam4)
    wr = np.zeros((128, 40), np.float32)
    for c4 in range(4):
        for m in range(10):
            r = m + 1
            if r < 10:
                wr[r * 4 + c4, m] = 1.0 if lam_eq else float(lam4[c4])
                wr[r * 4 + c4, 20 + m] = 1.0
            wr[m * 4 + c4, 10 + m] += 1.0
            wr[m * 4 + c4, 30 + m] += 1.0
            r2 = m + 2
            if r2 < 10:
                wr[r2 * 4 + c4, 10 + m] -= 1.0
                wr[r2 * 4 + c4, 30 + m] -= 1.0
    return wr


def _run(xs: np.ndarray, lam4, trace: bool = False, tmpdir=None):
    from concourse.bass_utils import run_bass_kernel_spmd

    nc = _get_nc(lam4)
    rwb = _runt_weights(lam4)
    in_maps = [{"x": np.ascontiguousarray(xs[PB * c:PB * (c + 1)]),
                "rw": rwb.copy()}
               for c in range(N_CORES)]
    res = run_bass_kernel_spmd(nc, in_maps, list(range(N_CORES)),
                               trace=trace, tmpdir=tmpdir)
    full = np.concatenate([res.results[c]["out"] for c in range(N_CORES)], axis=0)
    return full, res


def kernel(x, lam1x, lam2x, lam1y, lam2y):
    x = np.ascontiguousarray(np.asarray(x, dtype=np.float32))
    assert x.shape == (N, C, H, W), x.shape
    lam4 = np.asarray(lam1x, dtype=np.float32).reshape(-1)
    assert lam4.shape == (4,), lam4.shape
    full, _ = _run(x, lam4)
    return full



# revision 36
# speedup vs baseline: 1.0017x; 1.0017x over previous
"""Trainium2 Bass kernel for nn_Divergence2d.

Math (from the reference):
  q = C//4 = 4 channel groups A=x[:, :4], B=x[:,4:8], C=x[:,8:12], D=x[:,12:16]
  With per-group channel sums  A(r,c) = sum_ch lam_ch x[ch, r, c]  (lam only
  for group A) and a padded map  Gpad[r, c] = G[r-2, c-2]:

    out1[i,j] = (Apad[i+1, j+2] - Apad[i+1, j]) + Bpad[i, j+1] - Bpad[i+2, j+1]
    out2[i,j] = (Cpad[i+1, j+2] - Cpad[i+1, j]) + Dpad[i, j+1] - Dpad[i+2, j+1]

  for i,j in [0, 514)  (lam applied in the DVE combine when all lams equal).

Strategy: pure data parallel, 2 images per core on 8 cores.  Per image the
514 output rows are split into 4 full blocks of 126 plus a 10-row runt;
per full block ONE HWDGE DMA loads a row window of all 16 channels into an
SBUF tile [rows, 16ch x 512] (rhs pool 5-deep, so the load stream rides
out transient HBM-contention stalls).  The TensorE does the stencil via
stationary matrices (weights [window_row, out_row], channel sum by PSUM
accumulation over 4 chained matmuls).  All matmuls run in float32r mode;
the loose rel-err budget (2e-2) dwarfs the ~1e-3 rounding.

The runt uses a (row, channel)-packed tile (partition r*4+c4, one column
block per output map) with host-precomputed [128, 40] weights ("rw" input)
so its 320 KB spread over 10 DMA engines / 5 SBUF ports in one K=40 matmul
per map — a 10-partition layout would funnel through 2 SBUF AXI ports at
~54 GB/s and head-of-line block the load FIFO of engines 0-9 for ~6 us.
Image 0 processes it second (so block 0 leads the startup ramp), image 1
first (so the 4 full blocks stream uniformly into the drain).

Vertical conv padding is folded into the weights instead of rhs memsets:
  - block 0 loads x rows [0:128) at partition 0 and uses row-shifted
    weights (taps at negative x rows simply have no weight entry);
  - runt taps at x rows >= 512 have no weight entry in "rw".
ScalarE (ACT) drains PSUM into zero-padded SBUF staging tiles (pad columns
memset only on each staging buffer's first use); DVE does the combine ops;
per block the ch0 store rides the scalar/Act HWDGE ring and the ch1 store
the gpsimd SWDGE path — two independent trigger streams keep the write
traffic from backing up behind the ACT drains.  The final block's load is
group-split and its out2 combine + store row-split, shortening the
end-of-kernel drain.
"""
import sys

for _p in (
    "/root/.axon_site",
    "/root/.axon_site/_ro/trn_rl_repo",
    "/root/.axon_site/_ro/pypackages",
    "/opt/trn_rl_repo",
):
    if _p not in sys.path:
        sys.path.append(_p)

import numpy as np

N_CORES = 8
N, C, H, W = 16, 16, 512, 512
PB = N // N_CORES          # images per core
HO = WO = H + 2            # 514
CWPAD = 516                # staging width (2-col pad each side)
BLK = 126                  # output rows per block (matmul M)
BLOCKS = []
_i0 = 0
while _i0 < HO:
    BLOCKS.append((_i0, min(BLK, HO - _i0)))
    _i0 += BLK
# -> [(0,126), (126,126), (252,126), (378,126), (504,10)]

_cache = {}


def _build(lam4):
    import concourse.bacc as bacc
    import concourse.mybir as mybir
    from concourse.tile import TileContext

    f32 = mybir.dt.float32
    f32r = mybir.dt.float32r
    ALU = mybir.AluOpType
    ACT_COPY = mybir.ActivationFunctionType.Copy
    lam_eq = all(float(v) == float(lam4[0]) for v in lam4)

    nc = bacc.Bacc("TRN2", target_bir_lowering=False, debug=False,
                   num_devices=N_CORES, detect_race_conditions=False)
    x = nc.dram_tensor("x", (PB, C, H, W), f32, kind="ExternalInput")
    rw = nc.dram_tensor("rw", (128, 40), f32, kind="ExternalInput")
    out = nc.dram_tensor("out", (PB, 2, HO, WO), f32, kind="ExternalOutput")

    with TileContext(nc) as tc:
        with (
            tc.tile_pool(name="consts", bufs=1) as c_pool,
            tc.tile_pool(name="rhs", bufs=5) as rhs_pool,
            tc.tile_pool(name="rrhs", bufs=1) as rr_pool,
            tc.tile_pool(name="psum", bufs=2, space="PSUM") as ps_pool,
            tc.tile_pool(name="stage", bufs=2) as st_pool,
            tc.tile_pool(name="outs", bufs=3) as out_pool,
            tc.tile_pool(name="dtmp", bufs=2) as d_pool,
        ):
            # ---- one-time stencil weights [128 window rows, 126 out rows] --
            # Interior blocks (window row r = x row i0-2+r, out local m):
            #   S_s1[r, m] = d(r, m+1)           (A/C tap at x row i-1)
            #   S_bd[r, m] = d(r, m) - d(r, m+2) (B/D rows i-2 / i)
            # Block 0 (window row r = x row r):
            #   S_s1f[r, m] = d(r, m-1)
            #   S_bdf[r, m] = d(r, m-2) - d(r, m)
            def iota_t(tag, base, mult):
                t = c_pool.tile([128, BLK], f32, tag=tag, name=tag)
                nc.gpsimd.iota(t[:, :], pattern=[[0 if mult else 1, BLK]],
                               base=base, channel_multiplier=mult,
                               allow_small_or_imprecise_dtypes=True)
                return t

            R0 = iota_t("R0", 0, 1)          # r
            R1 = iota_t("R1", 1, 1)          # r + 1
            R2 = iota_t("R2", 2, 1)          # r + 2
            Sm0 = iota_t("Sm0", 0, 0)        # m
            Sm1 = iota_t("Sm1", 1, 0)        # m + 1
            Sm2 = iota_t("Sm2", 2, 0)        # m + 2

            def eq_t(tag, a, b, dt=None):
                t = c_pool.tile([128, BLK], dt or f32, tag=tag, name=tag)
                nc.vector.tensor_tensor(t[:, :], a[:, :], b[:, :], ALU.is_equal)
                return t

            # final weight tiles are float32r so their producing instruction
            # "rounds" them for the fp32r matmuls (values are exact anyway)
            S_s1 = eq_t("S_s1", R0, Sm1, f32r)
            e0 = eq_t("e0", R0, Sm0)         # (r == m)
            e2 = eq_t("e2", R0, Sm2)         # (r == m+2)
            S_bd = c_pool.tile([128, BLK], f32r, tag="S_bd")
            nc.vector.tensor_tensor(S_bd[:, :], e0[:, :], e2[:, :], ALU.subtract)
            S_s1f = eq_t("S_s1f", R1, Sm0, f32r)   # (r == m-1)
            e0f = eq_t("e0f", R2, Sm0)       # (r == m-2)
            S_bdf = c_pool.tile([128, BLK], f32r, tag="S_bdf")
            nc.vector.tensor_tensor(S_bdf[:, :], e0f[:, :], e0[:, :], ALU.subtract)

            if lam_eq:
                S_A_per_ch = [S_s1] * 4      # lam applied in the combine
                S_Af_per_ch = [S_s1f] * 4
            else:
                S_A_per_ch, S_Af_per_ch = [], []
                for c4 in range(4):
                    t = c_pool.tile([128, BLK], f32r, tag=f"S_A{c4}",
                                    name=f"S_A{c4}")
                    nc.vector.tensor_scalar_mul(t[:, :], S_s1[:, :],
                                                float(lam4[c4]))
                    S_A_per_ch.append(t)
                    tf = c_pool.tile([128, BLK], f32r, tag=f"S_Af{c4}",
                                     name=f"S_Af{c4}")
                    nc.vector.tensor_scalar_mul(tf[:, :], S_s1f[:, :],
                                                float(lam4[c4]))
                    S_Af_per_ch.append(tf)

            # runt weights, host-precomputed (see _runt_weights): one tile
            # [128, 40], rows c4*10+r, columns g*10+m (g = map A,B,C,D) —
            # a single K=40 matmul per output map.
            WRt = c_pool.tile([128, 40], f32r, tag="WRt", name="WRt")
            nc.sync.dma_start(out=WRt[:, :], in_=rw[:, :].bitcast(f32r))

            # ---- shared per-block tail: PSUM drain, combine, store ------
            def finish_block(n, i0, nr, ps, prime, tail_split=False):
                # ACT drains the A/C maps into zero-padded staging tiles
                # (pad columns memset only on each buffer's first use);
                # B/D are read directly from PSUM in the DVE combine
                st = {}
                for g in (0, 2):      # A/C: data at cols [2:514)
                    s = st_pool.tile([128, CWPAD], f32, tag=f"st{g}",
                                     name=f"st{g}")
                    st[g] = s
                    if prime:
                        nc.vector.memset(s[:, 0:2], 0.0)
                        nc.vector.memset(s[:, 514:CWPAD], 0.0)
                    nc.scalar.activation(s[0:nr, 2:514], ps[g][0:nr, :],
                                         ACT_COPY)
                # DVE combine
                o = out_pool.tile([128, 2 * WO], f32, tag="o")
                dA = d_pool.tile([128, WO], f32, tag="dA")
                nc.vector.tensor_tensor(dA[0:nr, :], st[0][0:nr, 2:2 + WO],
                                        st[0][0:nr, 0:WO], ALU.subtract)
                if lam_eq:
                    lam0 = float(lam4[0])
                    nc.vector.tensor_scalar_mul(o[0:nr, 0:1],
                                                dA[0:nr, 0:1], lam0)
                    nc.vector.tensor_scalar_mul(o[0:nr, 513:514],
                                                dA[0:nr, 513:514], lam0)
                    nc.vector.scalar_tensor_tensor(
                        o[0:nr, 1:513], dA[0:nr, 1:513], lam0,
                        ps[1][0:nr, :], ALU.mult, ALU.add)
                else:
                    nc.vector.tensor_scalar_mul(o[0:nr, 0:1],
                                                dA[0:nr, 0:1], 1.0)
                    nc.vector.tensor_scalar_mul(o[0:nr, 513:514],
                                                dA[0:nr, 513:514], 1.0)
                    nc.vector.tensor_tensor(o[0:nr, 1:513],
                                            dA[0:nr, 1:513],
                                            ps[1][0:nr, :], ALU.add)
                nc.vector.tensor_tensor(o[0:nr, WO:2 * WO],
                                        st[2][0:nr, 2:2 + WO],
                                        st[2][0:nr, 0:WO], ALU.subtract)
                nc.vector.tensor_tensor(o[0:nr, WO + 1:WO + 513],
                                        o[0:nr, WO + 1:WO + 513],
                                        ps[3][0:nr, :], ALU.add)
                # store: one DMA per channel; the (w2, w) split merges back
                # into one 2056B full-row packet per out row
                osrc = o[0:nr, :].rearrange("p (ch w2 w) -> p ch w2 w",
                                            w2=2, w=WO // 2)
                ov = out[n].rearrange("ch r (w2 w) -> ch r w2 w",
                                      w=WO // 2)
                for ch in range(2):
                    nc.scalar.dma_start(out=ov[ch, i0:i0 + nr, :, :],
                                        in_=osrc[:, ch, :, :])

            # The 10-row runt (out rows 504-513) loads in a
            # (row, channel)-packed layout: x[4g+c4, 502+r, :] lands on
            # partition r*4+c4, column block g.  Its 320 KB then spread
            # over 5 SBUF AXI ports and 10 DMA engines (descriptor groups
            # are assigned to engines by the source AP's outer dim = r)
            # instead of funneling through 2 ports, which head-of-line
            # blocked engines 0-9's load FIFO for ~6 us.  One K=40 matmul
            # per map replaces the 4-chained-matmul group.
            def runt_block(n, prime):
                rt = rr_pool.tile([128, 4 * 512], f32r, tag="rt")
                for g in range(4):
                    col = g * 512
                    nc.sync.dma_start(
                        out=rt[0:40, col:col + 512],
                        in_=x[n, 4 * g:4 * g + 4, 502:512, :].rearrange(
                            "c r w -> r c w").bitcast(f32r))
                i0r, nrr = BLOCKS[-1]
                ps = {}
                for g in range(4):
                    col = g * 512
                    p = ps_pool.tile([128, 512], f32, tag=f"ps{g}",
                                     name=f"ps{g}")
                    ps[g] = p
                    nc.tensor.matmul(p[0:nrr, :],
                                     WRt[0:40, g * 10:g * 10 + 10],
                                     rt[0:40, col:col + 512],
                                     start=True, stop=True)
                finish_block(n, i0r, nrr, ps, prime)

            # ---- main loop ---------------------------------------------
            # image 0: runt second, so block 0's load stream leads the
            # startup ramp; image 1: runt first, so the 4 full blocks
            # stream uniformly into the end-of-kernel drain.
            blk_idx = 0
            for n in range(PB):
                if n > 0:
                    blk_idx += 1
                    runt_block(n, blk_idx <= 2)

                # ---- 4 full blocks, uniform stream ----------------------
                for bi, (i0, nr) in enumerate(BLOCKS[:-1]):
                    blk_idx += 1
                    first = (i0 == 0)
                    if first:
                        rlo, K = 0, 128          # x rows [0:128) at part 0
                    else:
                        rlo = i0 - 2             # window row r = x row rlo+r
                        K = 128
                    t = rhs_pool.tile([128, 16 * 512], f32r, tag="rhs")
                    tv = t[:, :].rearrange("p (c w) -> p c w", w=512)
                    last_block = (n == PB - 1 and i0 == BLOCKS[-2][0])
                    if last_block:
                        # final processed block: split by channel groups so
                        # each map's matmuls start before the whole block is
                        # resident (shorter end-of-kernel drain).
                        # NOTE: splits must keep the full 128-partition extent
                        # (row-splits would engage only 2 of 16 SDMA engines)
                        nc.sync.dma_start(out=tv[0:K, 0:8, :],
                                          in_=x[n, 0:8, rlo:rlo + K, :].rearrange(
                                              "c r w -> r c w").bitcast(f32r))
                        nc.sync.dma_start(out=tv[0:K, 8:12, :],
                                          in_=x[n, 8:12, rlo:rlo + K, :].rearrange(
                                              "c r w -> r c w").bitcast(f32r))
                        nc.sync.dma_start(out=tv[0:K, 12:14, :],
                                          in_=x[n, 12:14, rlo:rlo + K, :].rearrange(
                                              "c r w -> r c w").bitcast(f32r))
                        nc.sync.dma_start(out=tv[0:K, 14:16, :],
                                          in_=x[n, 14:16, rlo:rlo + K, :].rearrange(
                                              "c r w -> r c w").bitcast(f32r))
                    elif blk_idx == 1:
                        # channel-wise head split: queues start draining after
                        # ~256 descriptors generate instead of 2048
                        nc.sync.dma_start(out=tv[0:K, 0:2, :],
                                          in_=x[n, 0:2, rlo:rlo + K, :].rearrange(
                                              "c r w -> r c w").bitcast(f32r))
                        nc.sync.dma_start(out=tv[0:K, 2:16, :],
                                          in_=x[n, 2:16, rlo:rlo + K, :].rearrange(
                                              "c r w -> r c w").bitcast(f32r))
                    else:
                        nc.sync.dma_start(out=tv[0:K, :, :],
                                          in_=x[n, :, rlo:rlo + K, :].rearrange(
                                              "c r w -> r c w").bitcast(f32r))
                    # group order A,B then C,D: the out1 combine only needs
                    # maps 0/1, so DVE overlaps the second half of the matmuls
                    if first:
                        groups = [(0, S_Af_per_ch), (1, [S_bdf] * 4),
                                  (2, [S_s1f] * 4), (3, [S_bdf] * 4)]
                    else:
                        groups = [(0, S_A_per_ch), (1, [S_bd] * 4),
                                  (2, [S_s1] * 4), (3, [S_bd] * 4)]
                    ps = {}
                    for g, weights in groups:
                        p = ps_pool.tile([128, 512], f32, tag=f"ps{g}",
                                         name=f"ps{g}")
                        ps[g] = p
                        for c4 in range(4):
                            ch = 4 * g + c4
                            nc.tensor.matmul(
                                p[0:BLK, :],
                                weights[c4][0:K, :],
                                t[0:K, 512 * ch:512 * ch + 512],
                                start=(c4 == 0), stop=(c4 == 3))
                    finish_block(n, i0, nr, ps, blk_idx <= 2,
                                 tail_split=last_block)
                    if n == 0 and bi == 0:
                        blk_idx += 1
                        runt_block(n, blk_idx <= 2)
    nc.finalize()
    return nc


def _get_nc(lam4):
    key = tuple(float(v) for v in lam4)
    if key not in _cache:
        _cache[key] = _build(key)
    return _cache[key]


def _runt_weights(lam4):
    """[128, 40] f32: runt stencil weights, rows r*4+c4, cols g*10+m.

    g = 0..3 -> maps A (lam-scaled unless all lams equal; then scaled in
    the combine), B, C, D.  Out row 504+m: A/C tap at x row 503+m
    (r = m+1), B/D taps +x[502+m] (r = m) and -x[504+m] (r = m+2); taps
    with r >= 10 (x rows >= 512) drop.
    """
    lam_eq = all(float(v) == float(lam4[0]) for v in lam4)
    wr = np.zeros((128, 40), np.float32)
    for c4 in range(4):
        for m in range(10):
            r = m + 1
            if r < 10:
                wr[r * 4 + c4, m] = 1.0 if lam_eq else float(lam4[c4])
                wr[r * 4 + c4, 20 + m] = 1.0
            wr[m * 4 + c4, 10 + m] += 1.0
            wr[m * 4 + c4, 30 + m] += 1.0
            r2 = m + 2
            if r2 < 10:
                wr[r2 * 4 + c4, 10 + m] -= 1.0
                wr[r2 * 4 + c4, 30 + m] -= 1.0
    return wr


def _run(xs: np.ndarray, lam4, trace: bool = False, tmpdir=None):
    from concourse.bass_utils import run_bass_kernel_spmd

    nc = _get_nc(lam4)
    rwb = _runt_weights(lam4)
    in_maps = [{"x": np.ascontiguousarray(xs[PB * c:PB * (c + 1)]),
                "rw": rwb.copy()}
               for c in range(N_CORES)]
    res = run_bass_kernel_spmd(nc, in_maps, list(range(N_CORES)),
                               trace=trace, tmpdir=tmpdir)
    full = np.concatenate([res.results[c]["out"] for c in range(N_CORES)], axis=0)
    return full, res


def kernel(x, lam1x, lam2x, lam1y, lam2y):
    x = np.ascontiguousarray(np.asarray(x, dtype=np.float32))
    assert x.shape == (N, C, H, W), x.shape
    lam4 = np.asarray(lam1x, dtype=np.float32).reshape(-1)
    assert lam4.shape == (4,), lam4.shape
    full, _ = _run(x, lam4)
    return full



# revision 37
# speedup vs baseline: 1.0200x; 1.0182x over previous
"""Trainium2 Bass kernel for nn_Divergence2d.

Math (from the reference):
  q = C//4 = 4 channel groups A=x[:, :4], B=x[:,4:8], C=x[:,8:12], D=x[:,12:16]
  With per-group channel sums  A(r,c) = sum_ch lam_ch x[ch, r, c]  (lam only
  for group A) and a padded map  Gpad[r, c] = G[r-2, c-2]:

    out1[i,j] = (Apad[i+1, j+2] - Apad[i+1, j]) + Bpad[i, j+1] - Bpad[i+2, j+1]
    out2[i,j] = (Cpad[i+1, j+2] - Cpad[i+1, j]) + Dpad[i, j+1] - Dpad[i+2, j+1]

  for i,j in [0, 514)  (lam applied in the DVE combine when all lams equal).

Strategy: pure data parallel, 2 images per core on 8 cores.  Per image the
514 output rows are split into 4 full blocks of 126 plus a 10-row runt;
per full block ONE HWDGE DMA loads a row window of all 16 channels into an
SBUF tile [rows, 16ch x 512] (rhs pool 5-deep, so the load stream rides
out transient HBM-contention stalls).  The TensorE does the stencil via
stationary matrices (weights [window_row, out_row], channel sum by PSUM
accumulation over 4 chained matmuls).  All matmuls run in float32r mode;
the loose rel-err budget (2e-2) dwarfs the ~1e-3 rounding.

The runt uses a (row, channel)-packed tile (partition r*4+c4, one column
block per output map) with host-precomputed [128, 40] weights ("rw" input)
so its 320 KB spread over 10 DMA engines / 5 SBUF ports in one K=40 matmul
per map — a 10-partition layout would funnel through 2 SBUF AXI ports at
~54 GB/s and head-of-line block the load FIFO of engines 0-9 for ~6 us.
Image 0 processes it second (so block 0 leads the startup ramp), image 1
first (so the 4 full blocks stream uniformly into the drain).

Vertical conv padding is folded into the weights instead of rhs memsets:
  - block 0 loads x rows [0:128) at partition 0 and uses row-shifted
    weights (taps at negative x rows simply have no weight entry);
  - runt taps at x rows >= 512 have no weight entry in "rw".
ScalarE (ACT) drains PSUM into zero-padded SBUF staging tiles (pad columns
memset only on each staging buffer's first use); DVE does the combine ops;
per block the ch0 store rides the scalar/Act HWDGE ring and the ch1 store
the gpsimd SWDGE path — two independent trigger streams keep the write
traffic from backing up behind the ACT drains.  The final block's load is
group-split and its out2 combine + store row-split, shortening the
end-of-kernel drain.
"""
import sys

for _p in (
    "/root/.axon_site",
    "/root/.axon_site/_ro/trn_rl_repo",
    "/root/.axon_site/_ro/pypackages",
    "/opt/trn_rl_repo",
):
    if _p not in sys.path:
        sys.path.append(_p)

import numpy as np

N_CORES = 8
N, C, H, W = 16, 16, 512, 512
PB = N // N_CORES          # images per core
HO = WO = H + 2            # 514
CWPAD = 516                # staging width (2-col pad each side)
BLK = 126                  # output rows per block (matmul M)
BLOCKS = []
_i0 = 0
while _i0 < HO:
    BLOCKS.append((_i0, min(BLK, HO - _i0)))
    _i0 += BLK
# -> [(0,126), (126,126), (252,126), (378,126), (504,10)]

_cache = {}


def _build(lam4):
    import concourse.bacc as bacc
    import concourse.mybir as mybir
    from concourse.tile import TileContext

    f32 = mybir.dt.float32
    f32r = mybir.dt.float32r
    ALU = mybir.AluOpType
    ACT_COPY = mybir.ActivationFunctionType.Copy
    lam_eq = all(float(v) == float(lam4[0]) for v in lam4)

    nc = bacc.Bacc("TRN2", target_bir_lowering=False, debug=False,
                   num_devices=N_CORES, detect_race_conditions=False)
    x = nc.dram_tensor("x", (PB, C, H, W), f32, kind="ExternalInput")
    rw = nc.dram_tensor("rw", (128, 40), f32, kind="ExternalInput")
    out = nc.dram_tensor("out", (PB, 2, HO, WO), f32, kind="ExternalOutput")

    with TileContext(nc) as tc:
        with (
            tc.tile_pool(name="consts", bufs=1) as c_pool,
            tc.tile_pool(name="rhs", bufs=5) as rhs_pool,
            tc.tile_pool(name="rrhs", bufs=1) as rr_pool,
            tc.tile_pool(name="psum", bufs=2, space="PSUM") as ps_pool,
            tc.tile_pool(name="stage", bufs=2) as st_pool,
            tc.tile_pool(name="outs", bufs=3) as out_pool,
            tc.tile_pool(name="dtmp", bufs=2) as d_pool,
        ):
            # ---- one-time stencil weights [128 window rows, 126 out rows] --
            # Interior blocks (window row r = x row i0-2+r, out local m):
            #   S_s1[r, m] = d(r, m+1)           (A/C tap at x row i-1)
            #   S_bd[r, m] = d(r, m) - d(r, m+2) (B/D rows i-2 / i)
            # Block 0 (window row r = x row r):
            #   S_s1f[r, m] = d(r, m-1)
            #   S_bdf[r, m] = d(r, m-2) - d(r, m)
            def iota_t(tag, base, mult):
                t = c_pool.tile([128, BLK], f32, tag=tag, name=tag)
                nc.gpsimd.iota(t[:, :], pattern=[[0 if mult else 1, BLK]],
                               base=base, channel_multiplier=mult,
                               allow_small_or_imprecise_dtypes=True)
                return t

            R0 = iota_t("R0", 0, 1)          # r
            R1 = iota_t("R1", 1, 1)          # r + 1
            R2 = iota_t("R2", 2, 1)          # r + 2
            Sm0 = iota_t("Sm0", 0, 0)        # m
            Sm1 = iota_t("Sm1", 1, 0)        # m + 1
            Sm2 = iota_t("Sm2", 2, 0)        # m + 2

            def eq_t(tag, a, b, dt=None):
                t = c_pool.tile([128, BLK], dt or f32, tag=tag, name=tag)
                nc.vector.tensor_tensor(t[:, :], a[:, :], b[:, :], ALU.is_equal)
                return t

            # final weight tiles are float32r so their producing instruction
            # "rounds" them for the fp32r matmuls (values are exact anyway)
            S_s1 = eq_t("S_s1", R0, Sm1, f32r)
            e0 = eq_t("e0", R0, Sm0)         # (r == m)
            e2 = eq_t("e2", R0, Sm2)         # (r == m+2)
            S_bd = c_pool.tile([128, BLK], f32r, tag="S_bd")
            nc.vector.tensor_tensor(S_bd[:, :], e0[:, :], e2[:, :], ALU.subtract)
            S_s1f = eq_t("S_s1f", R1, Sm0, f32r)   # (r == m-1)
            e0f = eq_t("e0f", R2, Sm0)       # (r == m-2)
            S_bdf = c_pool.tile([128, BLK], f32r, tag="S_bdf")
            nc.vector.tensor_tensor(S_bdf[:, :], e0f[:, :], e0[:, :], ALU.subtract)

            if lam_eq:
                S_A_per_ch = [S_s1] * 4      # lam applied in the combine
                S_Af_per_ch = [S_s1f] * 4
            else:
                S_A_per_ch, S_Af_per_ch = [], []
                for c4 in range(4):
                    t = c_pool.tile([128, BLK], f32r, tag=f"S_A{c4}",
                                    name=f"S_A{c4}")
                    nc.vector.tensor_scalar_mul(t[:, :], S_s1[:, :],
                                                float(lam4[c4]))
                    S_A_per_ch.append(t)
                    tf = c_pool.tile([128, BLK], f32r, tag=f"S_Af{c4}",
                                     name=f"S_Af{c4}")
                    nc.vector.tensor_scalar_mul(tf[:, :], S_s1f[:, :],
                                                float(lam4[c4]))
                    S_Af_per_ch.append(tf)


            # ---- shared per-block tail: PSUM drain, combine, store ------
            def finish_block(n, i0, nr, ps, prime, tail_split=False):
                # ACT drains the A/C maps into zero-padded staging tiles
                # (pad columns memset only on each buffer's first use);
                # B/D are read directly from PSUM in the DVE combine
                st = {}
                for g in (0, 2):      # A/C: data at cols [2:514)
                    s = st_pool.tile([128, CWPAD], f32, tag=f"st{g}",
                                     name=f"st{g}")
                    st[g] = s
                    if prime:
                        nc.vector.memset(s[:, 0:2], 0.0)
                        nc.vector.memset(s[:, 514:CWPAD], 0.0)
                    nc.scalar.activation(s[0:nr, 2:514], ps[g][0:nr, :],
                                         ACT_COPY)
                # DVE combine
                o = out_pool.tile([128, 2 * WO], f32, tag="o")
                dA = d_pool.tile([128, WO], f32, tag="dA")
                nc.vector.tensor_tensor(dA[0:nr, :], st[0][0:nr, 2:2 + WO],
                                        st[0][0:nr, 0:WO], ALU.subtract)
                if lam_eq:
                    lam0 = float(lam4[0])
                    nc.vector.tensor_scalar_mul(o[0:nr, 0:1],
                                                dA[0:nr, 0:1], lam0)
                    nc.vector.tensor_scalar_mul(o[0:nr, 513:514],
                                                dA[0:nr, 513:514], lam0)
                    nc.vector.scalar_tensor_tensor(
                        o[0:nr, 1:513], dA[0:nr, 1:513], lam0,
                        ps[1][0:nr, :], ALU.mult, ALU.add)
                else:
                    nc.vector.tensor_scalar_mul(o[0:nr, 0:1],
                                                dA[0:nr, 0:1], 1.0)
                    nc.vector.tensor_scalar_mul(o[0:nr, 513:514],
                                                dA[0:nr, 513:514], 1.0)
                    nc.vector.tensor_tensor(o[0:nr, 1:513],
                                            dA[0:nr, 1:513],
                                            ps[1][0:nr, :], ALU.add)
                nc.vector.tensor_tensor(o[0:nr, WO:2 * WO],
                                        st[2][0:nr, 2:2 + WO],
                                        st[2][0:nr, 0:WO], ALU.subtract)
                nc.vector.tensor_tensor(o[0:nr, WO + 1:WO + 513],
                                        o[0:nr, WO + 1:WO + 513],
                                        ps[3][0:nr, :], ALU.add)
                # store: one DMA per channel; the (w2, w) split merges back
                # into one 2056B full-row packet per out row
                osrc = o[0:nr, :].rearrange("p (ch w2 w) -> p ch w2 w",
                                            w2=2, w=WO // 2)
                ov = out[n].rearrange("ch r (w2 w) -> ch r w2 w",
                                      w=WO // 2)
                for ch in range(2):
                    nc.scalar.dma_start(out=ov[ch, i0:i0 + nr, :, :],
                                        in_=osrc[:, ch, :, :])

            # The 10-row runt (out rows 504-513) loads in a
            # (row, channel)-packed layout: x[4g+c4, 502+r, :] lands on
            # partition r*4+c4, column block g.  Its 320 KB then spread
            # over 5 SBUF AXI ports and 10 DMA engines (descriptor groups
            # are assigned to engines by the source AP's outer dim = r)
            # instead of funneling through 2 ports, which head-of-line
            # blocked engines 0-9's load FIFO for ~6 us.  One K=40 matmul
            # per map replaces the 4-chained-matmul group.
            def runt_block(n, prime):
                rt = rr_pool.tile([128, 4 * 512], f32r, tag="rt")
                for g in range(4):
                    col = g * 512
                    nc.sync.dma_start(
                        out=rt[0:40, col:col + 512],
                        in_=x[n, 4 * g:4 * g + 4, 502:512, :].rearrange(
                            "c r w -> r c w").bitcast(f32r))
                i0r, nrr = BLOCKS[-1]
                ps = {}
                for g in range(4):
                    col = g * 512
                    p = ps_pool.tile([128, 512], f32, tag=f"ps{g}",
                                     name=f"ps{g}")
                    ps[g] = p
                    nc.tensor.matmul(p[0:nrr, :],
                                     WRt[0:40, g * 10:g * 10 + 10],
                                     rt[0:40, col:col + 512],
                                     start=True, stop=True)
                finish_block(n, i0r, nrr, ps, prime)

            # ---- main loop ---------------------------------------------
            # image 0: runt second, so block 0's load stream leads the
            # startup ramp; image 1: runt first, so the 4 full blocks
            # stream uniformly into the end-of-kernel drain.
            blk_idx = 0
            for n in range(PB):
                if n > 0:
                    blk_idx += 1
                    runt_block(n, blk_idx <= 2)

                # ---- 4 full blocks, uniform stream ----------------------
                for bi, (i0, nr) in enumerate(BLOCKS[:-1]):
                    blk_idx += 1
                    first = (i0 == 0)
                    if first:
                        rlo, K = 0, 128          # x rows [0:128) at part 0
                    else:
                        rlo = i0 - 2             # window row r = x row rlo+r
                        K = 128
                    t = rhs_pool.tile([128, 16 * 512], f32r, tag="rhs")
                    tv = t[:, :].rearrange("p (c w) -> p c w", w=512)
                    last_block = (n == PB - 1 and i0 == BLOCKS[-2][0])
                    if last_block:
                        # final processed block: split by channel groups so
                        # each map's matmuls start before the whole block is
                        # resident (shorter end-of-kernel drain).
                        # NOTE: splits must keep the full 128-partition extent
                        # (row-splits would engage only 2 of 16 SDMA engines)
                        nc.sync.dma_start(out=tv[0:K, 0:8, :],
                                          in_=x[n, 0:8, rlo:rlo + K, :].rearrange(
                                              "c r w -> r c w").bitcast(f32r))
                        nc.sync.dma_start(out=tv[0:K, 8:12, :],
                                          in_=x[n, 8:12, rlo:rlo + K, :].rearrange(
                                              "c r w -> r c w").bitcast(f32r))
                        nc.sync.dma_start(out=tv[0:K, 12:14, :],
                                          in_=x[n, 12:14, rlo:rlo + K, :].rearrange(
                                              "c r w -> r c w").bitcast(f32r))
                        nc.sync.dma_start(out=tv[0:K, 14:16, :],
                                          in_=x[n, 14:16, rlo:rlo + K, :].rearrange(
                                              "c r w -> r c w").bitcast(f32r))
                    elif blk_idx == 1:
                        # channel-wise head split: queues start draining after
                        # ~128 descriptors generate instead of 2048
                        nc.sync.dma_start(out=tv[0:K, 0:1, :],
                                          in_=x[n, 0:1, rlo:rlo + K, :].rearrange(
                                              "c r w -> r c w").bitcast(f32r))
                        nc.sync.dma_start(out=tv[0:K, 1:16, :],
                                          in_=x[n, 1:16, rlo:rlo + K, :].rearrange(
                                              "c r w -> r c w").bitcast(f32r))
                    else:
                        nc.sync.dma_start(out=tv[0:K, :, :],
                                          in_=x[n, :, rlo:rlo + K, :].rearrange(
                                              "c r w -> r c w").bitcast(f32r))
                    # group order A,B then C,D: the out1 combine only needs
                    # maps 0/1, so DVE overlaps the second half of the matmuls
                    if first:
                        groups = [(0, S_Af_per_ch), (1, [S_bdf] * 4),
                                  (2, [S_s1f] * 4), (3, [S_bdf] * 4)]
                    else:
                        groups = [(0, S_A_per_ch), (1, [S_bd] * 4),
                                  (2, [S_s1] * 4), (3, [S_bd] * 4)]
                    ps = {}
                    for g, weights in groups:
                        p = ps_pool.tile([128, 512], f32, tag=f"ps{g}",
                                         name=f"ps{g}")
                        ps[g] = p
                        for c4 in range(4):
                            ch = 4 * g + c4
                            nc.tensor.matmul(
                                p[0:BLK, :],
                                weights[c4][0:K, :],
                                t[0:K, 512 * ch:512 * ch + 512],
                                start=(c4 == 0), stop=(c4 == 3))
                    finish_block(n, i0, nr, ps, blk_idx <= 2,
                                 tail_split=last_block)
                    if n == 0 and bi == 0:
                        # runt weights, host-precomputed (_runt_weights):
                        # [128, 40], rows r*4+c4, cols g*10+m — one K=40
                        # matmul per map.  Loaded here so its descriptors
                        # queue behind block 0's, not ahead of them.
                        WRt = c_pool.tile([128, 40], f32r, tag="WRt",
                                          name="WRt")
                        nc.sync.dma_start(out=WRt[:, :],
                                          in_=rw[:, :].bitcast(f32r))
                        blk_idx += 1
                        runt_block(n, blk_idx <= 2)
    nc.finalize()
    return nc


def _get_nc(lam4):
    key = tuple(float(v) for v in lam4)
    if key not in _cache:
        _cache[key] = _build(key)
    return _cache[key]


def _runt_weights(lam4):
    """[128, 40] f32: runt stencil weights, rows r*4+c4, cols g*10+m.

    g = 0..3 -> maps A (lam-scaled unless all lams equal; then scaled in
    the combine), B, C, D.  Out row 504+m: A/C tap at x row 503+m
    (r = m+1), B/D taps +x[502+m] (r = m) and -x[504+m] (r = m+2); taps
    with r >= 10 (x rows >= 512) drop.
    """
    lam_eq = all(float(v) == float(lam4[0]) for v in lam4)
    wr = np.zeros((128, 40), np.float32)
    for c4 in range(4):
        for m in range(10):
            r = m + 1
            if r < 10:
                wr[r * 4 + c4, m] = 1.0 if lam_eq else float(lam4[c4])
                wr[r * 4 + c4, 20 + m] = 1.0
            wr[m * 4 + c4, 10 + m] += 1.0
            wr[m * 4 + c4, 30 + m] += 1.0
            r2 = m + 2
            if r2 < 10:
                wr[r2 * 4 + c4, 10 + m] -= 1.0
                wr[r2 * 4 + c4, 30 + m] -= 1.0
    return wr


def _run(xs: np.ndarray, lam4, trace: bool = False, tmpdir=None):
    from concourse.bass_utils import run_bass_kernel_spmd

    nc = _get_nc(lam4)
    rwb = _runt_weights(lam4)
    in_maps = [{"x": np.ascontiguousarray(xs[PB * c:PB * (c + 1)]),
                "rw": rwb.copy()}
               for c in range(N_CORES)]
    res = run_bass_kernel_spmd(nc, in_maps, list(range(N_CORES)),
                               trace=trace, tmpdir=tmpdir)
    full = np.concatenate([res.results[c]["out"] for c in range(N_CORES)], axis=0)
    return full, res


def kernel(x, lam1x, lam2x, lam1y, lam2y):
    x = np.ascontiguousarray(np.asarray(x, dtype=np.float32))
    assert x.shape == (N, C, H, W), x.shape
    lam4 = np.asarray(lam1x, dtype=np.float32).reshape(-1)
    assert lam4.shape == (4,), lam4.shape
    full, _ = _run(x, lam4)
    return full

